# revision 8
# baseline (speedup 1.0000x reference)
"""MoE layer (8 experts, top-2 routing + shared expert) on 8 Trainium2 cores.

Strategy (expert parallelism per the sharding hint):
  - Host computes the router (logits -> softmax -> top-2 -> combine weights)
    and *dispatches*: core e receives the tokens routed to expert e (gathered,
    transposed to [D, C] layout, fp16) plus a 1/8 data-parallel slice of all
    tokens for the shared expert.
  - Each core runs one Bass/Tile kernel computing, for its token set,
      y = (silu(x @ Wg.T) * (x @ Wu.T)) @ Wd.T   (scaled by combine weight)
    for its expert's weights, then the same with the shared-expert weights.
    All matmuls are fp16 with fp32 PSUM accumulation.
  - Host *combines*: scatter-adds the per-expert outputs and the shared
    outputs back into the full [N, D] result.

Device layout per core (SPMD, one NEFF):
  xt  [D, TT]  fp16   tokens on the free dim, D on partitions (16 k-tiles)
  wg,wu [D, H] fp16   expert-then-shared weight loads (H on free dim)
  wd  [H, D]   fp16
  cw  [128, TT/128] f32  per-token combine weight, pre-grouped on host so
                         the DMA is contiguous (1.0 for the shared slice)
  y   [TT, D]  f32    output, tokens on partitions at write time

Pipeline per 512-token chunk: 2*11*16 matmuls produce g,u in PSUM per
128-row H tile; ScalarE applies Silu, VectorE multiplies into an fp16 act
tile [H, chunk]; 4x4x11 matmuls then contract act.T @ WdT into [128 tokens,
512 D] PSUM tiles, which VectorE scales by cw and DMAs out.
"""

import numpy as np
import ml_dtypes

import concourse.mybir as mybir
import concourse.tile as tile
from concourse import bacc
from concourse.bass import ds
from concourse.bass_utils import run_bass_kernel_spmd

P = 128
D = 2048
H = 1408
E = 8
TOP_K = 2
KD = D // P   # 16
KH = H // P   # 11
DT16 = mybir.dt.float16  # fp16: same PE rate as bf16, 8x the mantissa precision
F32 = mybir.dt.float32
F8 = mybir.dt.float8e4   # e4m3
E4M3 = ml_dtypes.float8_e4m3
SW = 32.0  # weight pre-scale before fp8 quantization (keeps W*32 ~ N(0,0.64))

# fp8 gate/up mode: x and Wg/Wu split into e4m3 (hi, lo); the three-term
# product x@W ~= xh@Wh + xl@Wh + xh@Wl runs as DoubleRow fp8 matmuls.
# "term": plain 3 sweeps (fastest if HW shadows DR weight loads)
# "kp": k-pair-outer with explicit ldweights reuse (amortizes weight loads)
FP8_GU = False
FP8_ORDER = "term"


def _chunks(count, base):
    """Split `count` tokens (multiple of 128) into chunks of 512 then 128."""
    out = []
    pos = 0
    while count - pos >= 512:
        out.append((base + pos, 512))
        pos += 512
    while count - pos >= P:
        out.append((base + pos, P))
        pos += P
    assert pos == count
    return out


def build_kernel(C, S, repeat=1, xb=2, ab=2, ob=2, pgu=3, pyb=2, wd_late=False, ysplit=False, tail_first=False, psg=None, psu=None, bulk_dma=True, store_eng="sync", y16=True, hsplit=4, gu_interleave=True, lead_split=0, fp8=None, fp8_order=None):
    """Build the SPMD Bass module for C expert tokens + S shared tokens."""
    if fp8 is None:
        fp8 = FP8_GU
    if fp8:
        return build_kernel_fp8(C, S, repeat=repeat, xb=xb, ab=ab, ob=ob,
                                pgu=pgu, pyb=pyb, y16=y16, hsplit=hsplit,
                                order=fp8_order or FP8_ORDER)
    TT = C + S
    assert C % P == 0 and S % P == 0

    nc = bacc.Bacc(
        "TRN2",
        target_bir_lowering=False,
        debug=False,
        enable_asserts=False,
        num_devices=8,
    )

    xt = nc.dram_tensor("xt", [D, TT], DT16, kind="ExternalInput").ap()
    wts = {}
    for pref in ("e", "s"):
        wts[pref] = (
            nc.dram_tensor(f"wg_{pref}", [D, H], DT16, kind="ExternalInput").ap(),
            nc.dram_tensor(f"wu_{pref}", [D, H], DT16, kind="ExternalInput").ap(),
            nc.dram_tensor(f"wd_{pref}", [H, D], DT16, kind="ExternalInput").ap(),
        )
    cw = nc.dram_tensor("cw", [P, TT // P], F32, kind="ExternalInput").ap()
    y = nc.dram_tensor("y", [TT, D], DT16 if y16 else F32, kind="ExternalOutput").ap()

    xt_r = xt.rearrange("(ko p) t -> p ko t", p=P)     # [128, 16, TT]
    y_r = y.rearrange("(g p) d -> p g d", p=P)         # [128, TT/128, 2048]
    cw_r = cw  # already [128, TT/128] host-transposed

    phases = [("e", 0, C), ("s", C, S)]

    with tile.TileContext(nc) as tc:
        with (
            tc.tile_pool(name="wgp", bufs=1) as wgp,
            tc.tile_pool(name="wup", bufs=1) as wup,
            tc.tile_pool(name="wdp", bufs=1) as wdp,
            tc.tile_pool(name="xp", bufs=xb) as xp,
            tc.tile_pool(name="ap", bufs=ab) as apool,
            tc.tile_pool(name="op", bufs=ob) as opool,
            tc.tile_pool(name="cp", bufs=1) as cpool,
            tc.tile_pool(name="psgu", bufs=pgu, space="PSUM") as psgu,
            tc.tile_pool(name="psgu2", bufs=(psu or pgu), space="PSUM") as psgu2,
            tc.tile_pool(name="psy", bufs=pyb, space="PSUM") as psy,
        ):
            cw_sb = cpool.tile([P, TT // P], F32)
            nc.sync.dma_start(cw_sb[:], cw_r)

            for pref, base, count in phases * repeat:
                if count == 0:
                    continue
                wg_d, wu_d, wd_d = wts[pref]
                wg_sb = wgp.tile([P, KD, H], DT16, tag="wg")
                wu_sb = wup.tile([P, KD, H], DT16, tag="wu")
                wg_rr = wg_d.rearrange("(ko p) h -> p ko h", p=P)
                wu_rr = wu_d.rearrange("(ko p) h -> p ko h", p=P)
                chunk_list = _chunks(count, base)
                if tail_first:
                    chunk_list = chunk_list[::-1]
                # bulk DMAs: a single large dma_start fans out across several
                # HW-DGE queues on real hardware (measured ~120us faster than
                # k-tile-split DMAs, even though the cost model says otherwise)
                start0, w0 = chunk_list[0]
                x0_sb = xp.tile([P, KD, 512], DT16, tag="x", name="x0_sb")[:, :, :w0]
                if bulk_dma:
                    if lead_split:
                        # DMAs are consumed roughly in issue order: put the
                        # pieces the first matmuls need first (x k-piece 1,
                        # wg/wu first h-piece), then the rest.
                        ksz = KD // lead_split
                        bounds = [H * i // hsplit for i in range(hsplit + 1)]
                        bounds = [(b // P) * P for b in bounds]
                        bounds[-1] = H
                        bounds = [0, 2 * P] + [b for b in bounds[1:] if b > 2 * P]
                        nc.sync.dma_start(
                            x0_sb[:, 0:ksz, :], xt_r[:, 0:ksz, ds(start0, w0)]
                        )
                        nc.sync.dma_start(wg_sb[:, :, : bounds[1]], wg_rr[:, :, : bounds[1]])
                        nc.sync.dma_start(wu_sb[:, :, : bounds[1]], wu_rr[:, :, : bounds[1]])
                        for k0 in range(ksz, KD, ksz):
                            nc.sync.dma_start(
                                x0_sb[:, k0 : k0 + ksz, :],
                                xt_r[:, k0 : k0 + ksz, ds(start0, w0)],
                            )
                        for h0, h1 in zip(bounds[1:-1], bounds[2:]):
                            nc.sync.dma_start(wg_sb[:, :, h0:h1], wg_rr[:, :, h0:h1])
                            nc.sync.dma_start(wu_sb[:, :, h0:h1], wu_rr[:, :, h0:h1])
                    else:
                        nc.sync.dma_start(x0_sb[:], xt_r[:, :, ds(start0, w0)])
                        if hsplit > 1:
                            # split along H so early h-tiles' weights land first;
                            # pieces stay >=1.4MB for multi-queue DMA fanout
                            bounds = [H * i // hsplit for i in range(hsplit + 1)]
                            bounds = [(b // P) * P for b in bounds]
                            bounds[-1] = H
                            for h0, h1 in zip(bounds[:-1], bounds[1:]):
                                nc.sync.dma_start(wg_sb[:, :, h0:h1], wg_rr[:, :, h0:h1])
                                nc.sync.dma_start(wu_sb[:, :, h0:h1], wu_rr[:, :, h0:h1])
                        else:
                            nc.sync.dma_start(wg_sb[:], wg_rr)
                            nc.sync.dma_start(wu_sb[:], wu_rr)
                else:
                    for k in range(KD):
                        nc.sync.dma_start(x0_sb[:, k, :], xt_r[:, k, ds(start0, w0)])
                        nc.sync.dma_start(wg_sb[:, k, :], wg_rr[:, k, :])
                        nc.sync.dma_start(wu_sb[:, k, :], wu_rr[:, k, :])
                wd_sb = wdp.tile([P, KH, D], DT16, tag="wd")
                wd_rr = wd_d.rearrange("(ho p) d -> p ho d", p=P)
                if bulk_dma:
                    nc.sync.dma_start(wd_sb[:], wd_rr)
                elif not wd_late:
                    for h in range(KH):
                        nc.sync.dma_start(wd_sb[:, h, :], wd_rr[:, h, :])

                for ci, (start, w) in enumerate(chunk_list):
                    if wd_late and ci == 1:
                        for h in range(KH):
                            nc.sync.dma_start(wd_sb[:, h, :], wd_rr[:, h, :])
                    if ci == 0:
                        x_sb = x0_sb
                    else:
                        x_sb = xp.tile([P, KD, 512], DT16, tag="x", name="x_sb")[:, :, :w]
                        if bulk_dma:
                            nc.sync.dma_start(x_sb, xt_r[:, :, ds(start, w)])
                        else:
                            for k in range(KD):
                                nc.sync.dma_start(x_sb[:, k, :], xt_r[:, k, ds(start, w)])

                    aT = apool.tile([P, KH, 512], DT16, tag="a", name="aT")[:, :, :w]
                    for h in range(KH):
                        pg = psgu.tile([P, 512], F32, tag="psg", name="pg")[:, :w]
                        pu = psgu2.tile([P, 512], F32, tag="psu", name="pu")[:, :w]
                        if gu_interleave:
                            for k in range(KD):
                                nc.tensor.matmul(
                                    pg, wg_sb[:, k, h * P : (h + 1) * P], x_sb[:, k, :],
                                    start=(k == 0), stop=(k == KD - 1),
                                )
                                nc.tensor.matmul(
                                    pu, wu_sb[:, k, h * P : (h + 1) * P], x_sb[:, k, :],
                                    start=(k == 0), stop=(k == KD - 1),
                                )
                        else:
                            for k in range(KD):
                                nc.tensor.matmul(
                                    pg, wg_sb[:, k, h * P : (h + 1) * P], x_sb[:, k, :],
                                    start=(k == 0), stop=(k == KD - 1),
                                )
                            for k in range(KD):
                                nc.tensor.matmul(
                                    pu, wu_sb[:, k, h * P : (h + 1) * P], x_sb[:, k, :],
                                    start=(k == 0), stop=(k == KD - 1),
                                )
                        nc.scalar.activation(
                            aT[:, h, :], pg, mybir.ActivationFunctionType.Silu
                        )
                        nc.vector.tensor_tensor(
                            aT[:, h, :], aT[:, h, :], pu, mybir.AluOpType.mult
                        )

                    for g in range(w // P):
                        gg = (start + g * P) // P
                        out_sb = opool.tile([P, 4, 512], DT16 if y16 else F32, tag="o", name="out_sb")
                        for d4 in range(4):
                            py = psy.tile([P, 512], F32, tag="psy", name="py")
                            for h in range(KH):
                                nc.tensor.matmul(
                                    py,
                                    aT[:, h, g * P : (g + 1) * P],
                                    wd_sb[:, h, d4 * 512 : (d4 + 1) * 512],
                                    start=(h == 0),
                                    stop=(h == KH - 1),
                                )
                            nc.vector.tensor_scalar_mul(
                                out_sb[:, d4, :], py, cw_sb[:, gg : gg + 1]
                            )
                        se = nc.scalar if store_eng == "scalar" else nc.sync
                        if ysplit:
                            for d4 in range(4):
                                se.dma_start(
                                    y_r[:, gg, d4 * 512 : (d4 + 1) * 512],
                                    out_sb[:, d4, :],
                                )
                        else:
                            se.dma_start(y_r[:, gg, :], out_sb[:])

    nc.compile()
    return nc


def build_kernel_fp8(C, S, repeat=1, xb=2, ab=2, ob=2, pgu=3, pyb=2, y16=True,
                     hsplit=4, order="term"):
    """fp8 gate/up variant: x and Wg/Wu arrive as e4m3 (hi, lo) pairs; the
    gate/up matmuls run as 3-term DoubleRow fp8 (xh@Wh + xl@Wh + xh@Wl) at
    2x contraction per instruction. Scales: x*1, W*32 -> psum = 32*(x@W.T);
    silu applies scale=1/32; aT = silu(g) * (32*u) = 32*act; Wd is host-
    divided by 32 so the down matmul and cw path are unchanged."""
    TT = C + S
    assert C % P == 0 and S % P == 0
    KP = KD // 2  # 8 DoubleRow k-pairs

    nc = bacc.Bacc(
        "TRN2",
        target_bir_lowering=False,
        debug=False,
        enable_asserts=False,
        num_devices=8,
    )

    xth = nc.dram_tensor("xth", [D, TT], F8, kind="ExternalInput").ap()
    xtl = nc.dram_tensor("xtl", [D, TT], F8, kind="ExternalInput").ap()
    wts = {}
    for pref in ("e", "s"):
        wts[pref] = (
            nc.dram_tensor(f"wgh_{pref}", [D, H], F8, kind="ExternalInput").ap(),
            nc.dram_tensor(f"wgl_{pref}", [D, H], F8, kind="ExternalInput").ap(),
            nc.dram_tensor(f"wuh_{pref}", [D, H], F8, kind="ExternalInput").ap(),
            nc.dram_tensor(f"wul_{pref}", [D, H], F8, kind="ExternalInput").ap(),
            nc.dram_tensor(f"wd_{pref}", [H, D], DT16, kind="ExternalInput").ap(),
        )
    cw = nc.dram_tensor("cw", [P, TT // P], F32, kind="ExternalInput").ap()
    y = nc.dram_tensor("y", [TT, D], DT16 if y16 else F32, kind="ExternalOutput").ap()

    xth_r = xth.rearrange("(ko p) t -> p ko t", p=P)   # [128, 16, TT]
    xtl_r = xtl.rearrange("(ko p) t -> p ko t", p=P)
    y_r = y.rearrange("(g p) d -> p g d", p=P)         # [128, TT/128, 2048]

    phases = [("e", 0, C), ("s", C, S)]
    DR = mybir.MatmulPerfMode.DoubleRow

    with tile.TileContext(nc) as tc:
        with (
            tc.tile_pool(name="wghp", bufs=1) as wghp,
            tc.tile_pool(name="wglp", bufs=1) as wglp,
            tc.tile_pool(name="wuhp", bufs=1) as wuhp,
            tc.tile_pool(name="wulp", bufs=1) as wulp,
            tc.tile_pool(name="wdp", bufs=1) as wdp,
            tc.tile_pool(name="xhp", bufs=xb) as xhp,
            tc.tile_pool(name="xlp", bufs=xb) as xlp,
            tc.tile_pool(name="ap", bufs=ab) as apool,
            tc.tile_pool(name="op", bufs=ob) as opool,
            tc.tile_pool(name="cp", bufs=1) as cpool,
            tc.tile_pool(name="psgu", bufs=pgu, space="PSUM") as psgu,
            tc.tile_pool(name="psgu2", bufs=pgu, space="PSUM") as psgu2,
            tc.tile_pool(name="psy", bufs=pyb, space="PSUM") as psy,
        ):
            cw_sb = cpool.tile([P, TT // P], F32)
            nc.sync.dma_start(cw_sb[:], cw)

            for pref, base, count in phases * repeat:
                if count == 0:
                    continue
                wgh_d, wgl_d, wuh_d, wul_d, wd_d = wts[pref]
                wgh_sb = wghp.tile([P, KD, H], F8, tag="wgh")
                wgl_sb = wglp.tile([P, KD, H], F8, tag="wgl")
                wuh_sb = wuhp.tile([P, KD, H], F8, tag="wuh")
                wul_sb = wulp.tile([P, KD, H], F8, tag="wul")
                w_rr = [t.rearrange("(ko p) h -> p ko h", p=P)
                        for t in (wgh_d, wgl_d, wuh_d, wul_d)]
                w_sb = [wgh_sb, wgl_sb, wuh_sb, wul_sb]
                chunk_list = _chunks(count, base)
                start0, w0 = chunk_list[0]
                xh0_sb = xhp.tile([P, KD, 512], F8, tag="xh", name="xh0_sb")[:, :, :w0]
                xl0_sb = xlp.tile([P, KD, 512], F8, tag="xl", name="xl0_sb")[:, :, :w0]
                nc.sync.dma_start(xh0_sb[:], xth_r[:, :, ds(start0, w0)])
                nc.sync.dma_start(xl0_sb[:], xtl_r[:, :, ds(start0, w0)])
                bounds = [H * i // hsplit for i in range(hsplit + 1)]
                bounds = [(b // P) * P for b in bounds]
                bounds[-1] = H
                for h0, h1 in zip(bounds[:-1], bounds[1:]):
                    for sb, rr in zip(w_sb, w_rr):
                        nc.sync.dma_start(sb[:, :, h0:h1], rr[:, :, h0:h1])
                wd_sb = wdp.tile([P, KH, D], DT16, tag="wd")
                wd_rr = wd_d.rearrange("(ho p) d -> p ho d", p=P)
                nc.sync.dma_start(wd_sb[:], wd_rr)

                for ci, (start, w) in enumerate(chunk_list):
                    if ci == 0:
                        xh_sb, xl_sb = xh0_sb, xl0_sb
                    else:
                        xh_sb = xhp.tile([P, KD, 512], F8, tag="xh", name="xh_sb")[:, :, :w]
                        xl_sb = xlp.tile([P, KD, 512], F8, tag="xl", name="xl_sb")[:, :, :w]
                        nc.sync.dma_start(xh_sb, xth_r[:, :, ds(start, w)])
                        nc.sync.dma_start(xl_sb, xtl_r[:, :, ds(start, w)])

                    aT = apool.tile([P, KH, 512], DT16, tag="a", name="aT")[:, :, :w]
                    for h in range(KH):
                        pg = psgu.tile([P, 512], F32, tag="psg", name="pg")[:, :w]
                        pu = psgu2.tile([P, 512], F32, tag="psu", name="pu")[:, :w]
                        hs = slice(h * P, (h + 1) * P)
                        if order == "term":
                            # 3 DoubleRow sweeps per matrix, g/u interleaved
                            terms_g = [(xh_sb, wgh_sb), (xl_sb, wgh_sb), (xh_sb, wgl_sb)]
                            terms_u = [(xh_sb, wuh_sb), (xl_sb, wuh_sb), (xh_sb, wul_sb)]
                            for t in range(3):
                                xg, wg_ = terms_g[t]
                                xu, wu_ = terms_u[t]
                                for kp in range(KP):
                                    ks = slice(2 * kp, 2 * kp + 2)
                                    nc.tensor.matmul(
                                        pg, wg_[:, ks, hs], xg[:, ks, :],
                                        start=(t == 0 and kp == 0),
                                        stop=(t == 2 and kp == KP - 1),
                                        perf_mode=DR,
                                    )
                                    nc.tensor.matmul(
                                        pu, wu_[:, ks, hs], xu[:, ks, :],
                                        start=(t == 0 and kp == 0),
                                        stop=(t == 2 and kp == KP - 1),
                                        perf_mode=DR,
                                    )
                        else:  # "kp": explicit ldweights, stationary reused
                            for kp in range(KP):
                                ks = slice(2 * kp, 2 * kp + 2)
                                for ps, whi, wlo in (
                                    (pg, wgh_sb, wgl_sb),
                                    (pu, wuh_sb, wul_sb),
                                ):
                                    nc.tensor.ldweights(whi[:, ks, hs], perf_mode=DR)
                                    for i, xs in enumerate((xh_sb, xl_sb)):
                                        mm = nc.tensor.matmul(
                                            ps, whi[:, ks, hs], xs[:, ks, :],
                                            start=(kp == 0 and i == 0),
                                            stop=False,
                                            perf_mode=DR,
                                        )
                                        mm.ldweights = False
                                    nc.tensor.ldweights(wlo[:, ks, hs], perf_mode=DR)
                                    mm = nc.tensor.matmul(
                                        ps, wlo[:, ks, hs], xh_sb[:, ks, :],
                                        start=False,
                                        stop=(kp == KP - 1),
                                        perf_mode=DR,
                                    )
                                    mm.ldweights = False
                        nc.scalar.activation(
                            aT[:, h, :], pg, mybir.ActivationFunctionType.Silu,
                            scale=1.0 / SW,
                        )
                        nc.vector.tensor_tensor(
                            aT[:, h, :], aT[:, h, :], pu, mybir.AluOpType.mult
                        )

                    for g in range(w // P):
                        gg = (start + g * P) // P
                        out_sb = opool.tile([P, 4, 512], DT16 if y16 else F32, tag="o", name="out_sb")
                        for d4 in range(4):
                            py = psy.tile([P, 512], F32, tag="psy", name="py")
                            for h in range(KH):
                                nc.tensor.matmul(
                                    py,
                                    aT[:, h, g * P : (g + 1) * P],
                                    wd_sb[:, h, d4 * 512 : (d4 + 1) * 512],
                                    start=(h == 0),
                                    stop=(h == KH - 1),
                                )
                            nc.vector.tensor_scalar_mul(
                                out_sb[:, d4, :], py, cw_sb[:, gg : gg + 1]
                            )
                        nc.sync.dma_start(y_r[:, gg, :], out_sb[:])

    nc.compile()
    return nc


def _route(x_flat, gate_w, expert_bias):
    """Replicate the reference router in numpy (fp32)."""
    N = x_flat.shape[0]
    logits = x_flat @ gate_w.T                       # [N, E]
    m = logits.max(-1, keepdims=True)
    p = np.exp(logits - m)
    p /= p.sum(-1, keepdims=True)
    biased = logits + expert_bias
    rows = np.arange(N)
    i1 = biased.argmax(-1)
    b2 = biased.copy()
    b2[rows, i1] = -np.inf
    i2 = b2.argmax(-1)
    w1 = p[rows, i1]
    w2 = p[rows, i2]
    s = w1 + w2
    return i1, i2, w1 / s, w2 / s


def _prepare(inputs):
    x = np.asarray(inputs["x"], dtype=np.float32)
    B, S_, D_ = x.shape
    assert D_ == D
    x_flat = x.reshape(-1, D)
    N = x_flat.shape[0]
    S0 = N // 8

    i1, i2, w1, w2 = _route(
        x_flat,
        np.asarray(inputs["gate_w"], dtype=np.float32),
        np.asarray(inputs["expert_bias"], dtype=np.float32),
    )

    idx_lists = []
    w_lists = []
    for e in range(E):
        m1 = i1 == e
        m2 = i2 == e
        idx = np.nonzero(m1 | m2)[0]
        w = np.where(m1[idx], w1[idx], w2[idx]).astype(np.float32)
        idx_lists.append(idx)
        w_lists.append(w)

    maxc = max(len(ix) for ix in idx_lists)
    C = ((maxc + P - 1) // P) * P
    TT = C + S0

    bf = np.float16
    Wg = np.asarray(inputs["Wg"], dtype=np.float32)
    Wu = np.asarray(inputs["Wu"], dtype=np.float32)
    Wd = np.asarray(inputs["Wd"], dtype=np.float32)
    fp8 = FP8_GU

    def split8(a):
        """fp32 array -> (hi, lo) e4m3 pair with hi + lo ~= a."""
        hi = a.astype(E4M3)
        lo = (a - hi.astype(np.float32)).astype(E4M3)
        return hi, lo

    if fp8:
        xfh, xfl = split8(x_flat)

    def gu_weights(wgT, wuT, wdT, pref):
        """Per-phase weight dict from [D,H] gate/up and [H,D] down (fp32)."""
        if not fp8:
            return {
                f"wg_{pref}": np.ascontiguousarray(wgT.T).astype(bf),
                f"wu_{pref}": np.ascontiguousarray(wuT.T).astype(bf),
                f"wd_{pref}": np.ascontiguousarray(wdT.T).astype(bf),
            }
        wgh, wgl = split8(np.ascontiguousarray(wgT.T) * SW)
        wuh, wul = split8(np.ascontiguousarray(wuT.T) * SW)
        return {
            f"wgh_{pref}": wgh, f"wgl_{pref}": wgl,
            f"wuh_{pref}": wuh, f"wul_{pref}": wul,
            f"wd_{pref}": (np.ascontiguousarray(wdT.T) / SW).astype(bf),
        }

    sw = gu_weights(
        np.asarray(inputs["Ws_g"], np.float32),
        np.asarray(inputs["Ws_u"], np.float32),
        np.asarray(inputs["Ws_d"], np.float32),
        "s",
    )

    in_maps = []
    idx_pad = np.empty((E, C), dtype=np.int64)
    for e in range(E):
        idx = idx_lists[e]
        pad = np.full(C - len(idx), N, dtype=np.int64)  # N -> dummy row
        idx_pad[e] = np.concatenate([idx, pad])
        gather_idx = np.concatenate([idx, np.zeros(C - len(idx), np.int64)])

        cwv = np.ones(TT, dtype=np.float32)
        cwv[: len(idx)] = w_lists[e]
        cwv[len(idx) : C] = 0.0
        cwv = np.ascontiguousarray(cwv.reshape(TT // P, P).T)

        im = {"cw": cwv}
        if fp8:
            for nm, src in (("xth", xfh), ("xtl", xfl)):
                xt = np.empty((D, TT), dtype=E4M3)
                xt[:, :C] = src[gather_idx].T
                xt[:, C:] = src[e * S0 : (e + 1) * S0].T
                im[nm] = xt
        else:
            xt = np.empty((D, TT), dtype=bf)
            xt[:, :C] = x_flat[gather_idx].T
            xt[:, C:] = x_flat[e * S0 : (e + 1) * S0].T
            im["xt"] = xt
        im.update(gu_weights(Wg[e], Wu[e], Wd[e], "e"))
        im.update(sw)
        in_maps.append(im)
    return x, in_maps, idx_pad, C, S0, N


def _combine(x_shape, results, idx_pad, C, S0, N):
    acc = np.zeros((N + 1, D), dtype=np.float32)
    for e in range(E):
        ye = results[e]["y"]
        acc[idx_pad[e]] += ye[:C]
        acc[e * S0 : (e + 1) * S0] += ye[C:]
    return acc[:N].reshape(x_shape)


def kernel(**inputs) -> np.ndarray:
    x, in_maps, idx_pad, C, S0, N = _prepare(inputs)
    nc = build_kernel(C, S0)
    res = run_bass_kernel_spmd(nc, in_maps, core_ids=list(range(8)))
    return _combine(x.shape, [res.results[e] for e in range(E)], idx_pad, C, S0, N)



# revision 16
# speedup vs baseline: 1.3673x; 1.3673x over previous
"""MoE layer (8 experts, top-2 routing + shared expert) on 8 Trainium2 cores.

Strategy (expert parallelism per the sharding hint):
  - Host computes the router (logits -> softmax -> top-2 -> combine weights)
    and *dispatches*: core e receives the tokens routed to expert e (gathered,
    transposed to [D, C] layout, fp16) plus a 1/8 data-parallel slice of all
    tokens for the shared expert.
  - Each core runs one Bass/Tile kernel computing, for its token set,
      y = (silu(x @ Wg.T) * (x @ Wu.T)) @ Wd.T   (scaled by combine weight)
    for its expert's weights, then the same with the shared-expert weights.
    All matmuls are fp16 with fp32 PSUM accumulation.
  - Host *combines*: scatter-adds the per-expert outputs and the shared
    outputs back into the full [N, D] result.

Device layout per core (SPMD, one NEFF):
  xt  [D, TT]  fp16   tokens on the free dim, D on partitions (16 k-tiles)
  wg,wu [D, H] fp16   expert-then-shared weight loads (H on free dim)
  wd  [H, D]   fp16
  cw  [128, TT/128] f32  per-token combine weight, pre-grouped on host so
                         the DMA is contiguous (1.0 for the shared slice)
  y   [TT, D]  f32    output, tokens on partitions at write time

Pipeline per 512-token chunk: 2*11*16 matmuls produce g,u in PSUM per
128-row H tile; ScalarE applies Silu, VectorE multiplies into an fp16 act
tile [H, chunk]; 4x4x11 matmuls then contract act.T @ WdT into [128 tokens,
512 D] PSUM tiles, which VectorE scales by cw and DMAs out.
"""

import numpy as np
import ml_dtypes

import concourse.mybir as mybir
import concourse.tile as tile
from concourse import bacc
from concourse.bass import ds
from concourse.bass_utils import run_bass_kernel_spmd

P = 128
D = 2048
H = 1408
E = 8
TOP_K = 2
KD = D // P   # 16
KH = H // P   # 11
DT16 = mybir.dt.float16  # fp16: same PE rate as bf16, 8x the mantissa precision
F32 = mybir.dt.float32
F8 = mybir.dt.float8e4   # e4m3
E4M3 = ml_dtypes.float8_e4m3
SW = 32.0  # weight pre-scale before fp8 quantization (keeps W*32 ~ N(0,0.64))

# fp8 gate/up mode: x and Wg/Wu split into e4m3 (hi, lo); the three-term
# product x@W ~= xh@Wh + xl@Wh + xh@Wl runs as DoubleRow fp8 matmuls.
# "term": plain 3 sweeps (fastest if HW shadows DR weight loads)
# "kp": k-pair-outer with explicit ldweights reuse (amortizes weight loads)
FP8_GU = False
FP8_ORDER = "term"


def _chunks(count, base):
    """Split `count` tokens (multiple of 64) into chunks of 512, then 128,
    then one final 64 — minimizing chunk count (g/u instruction count is
    per-chunk, independent of width)."""
    out = []
    pos = 0
    while count - pos >= 512:
        out.append((base + pos, 512))
        pos += 512
    while count - pos >= P:
        out.append((base + pos, P))
        pos += P
    if count - pos == 64:
        out.append((base + pos, 64))
        pos += 64
    assert pos == count
    return out


def _ceil(v, m):
    return -(-v // m) * m


def build_kernel(C, S, repeat=1, xb=2, ab=2, ob=2, pgu=3, pyb=2, wd_late=False, ysplit=False, tail_first=False, psg=None, psu=None, bulk_dma=True, store_eng="sync", y16=True, hsplit=4, gu_interleave=True, lead_split=0, fp8=None, fp8_order=None):
    """Build the SPMD Bass module for C expert tokens + S shared tokens."""
    if fp8 is None:
        fp8 = FP8_GU
    if fp8:
        return build_kernel_fp8(C, S, repeat=repeat, xb=xb, ab=ab, ob=ob,
                                pgu=pgu, pyb=pyb, y16=y16, hsplit=hsplit,
                                order=fp8_order or FP8_ORDER)
    TT = C + S
    TT_pad = _ceil(TT, P)  # dram layouts need 128-row groups; compute uses TT
    assert C % 64 == 0 and S % P == 0

    nc = bacc.Bacc(
        "TRN2",
        target_bir_lowering=False,
        debug=False,
        enable_asserts=False,
        num_devices=8,
    )

    xt = nc.dram_tensor("xt", [D, TT_pad], DT16, kind="ExternalInput").ap()
    wts = {}
    for pref in ("e", "s"):
        wts[pref] = (
            nc.dram_tensor(f"wg_{pref}", [D, H], DT16, kind="ExternalInput").ap(),
            nc.dram_tensor(f"wu_{pref}", [D, H], DT16, kind="ExternalInput").ap(),
            nc.dram_tensor(f"wd_{pref}", [H, D], DT16, kind="ExternalInput").ap(),
        )
    cw = nc.dram_tensor("cw", [P, TT_pad // P], F32, kind="ExternalInput").ap()
    y = nc.dram_tensor("y", [TT_pad, D], DT16 if y16 else F32, kind="ExternalOutput").ap()

    xt_r = xt.rearrange("(ko p) t -> p ko t", p=P)     # [128, 16, TT_pad]
    y_r = y.rearrange("(g p) d -> p g d", p=P)         # [128, TT_pad/128, 2048]
    cw_r = cw  # already [128, TT_pad/128] host-transposed

    # shared phase first (always 128-aligned); expert phase second so its
    # possibly-partial final 64-token group is the last group of the body
    phases = [("s", 0, S), ("e", S, C)]

    with tile.TileContext(nc) as tc:
        with (
            tc.tile_pool(name="wgp", bufs=1) as wgp,
            tc.tile_pool(name="wup", bufs=1) as wup,
            tc.tile_pool(name="wdp", bufs=1) as wdp,
            tc.tile_pool(name="xp", bufs=xb) as xp,
            tc.tile_pool(name="ap", bufs=ab) as apool,
            tc.tile_pool(name="op", bufs=ob) as opool,
            tc.tile_pool(name="cp", bufs=1) as cpool,
            tc.tile_pool(name="psgu", bufs=pgu, space="PSUM") as psgu,
            tc.tile_pool(name="psgu2", bufs=(psu or pgu), space="PSUM") as psgu2,
            tc.tile_pool(name="psy", bufs=pyb, space="PSUM") as psy,
        ):
            cw_sb = cpool.tile([P, TT_pad // P], F32)
            nc.sync.dma_start(cw_sb[:], cw_r)

            for pref, base, count in phases * repeat:
                if count == 0:
                    continue
                wg_d, wu_d, wd_d = wts[pref]
                wg_sb = wgp.tile([P, KD, H], DT16, tag="wg")
                wu_sb = wup.tile([P, KD, H], DT16, tag="wu")
                wg_rr = wg_d.rearrange("(ko p) h -> p ko h", p=P)
                wu_rr = wu_d.rearrange("(ko p) h -> p ko h", p=P)
                chunk_list = _chunks(count, base)
                if tail_first:
                    chunk_list = chunk_list[::-1]
                # bulk DMAs: a single large dma_start fans out across several
                # HW-DGE queues on real hardware (measured ~120us faster than
                # k-tile-split DMAs, even though the cost model says otherwise)
                start0, w0 = chunk_list[0]
                x0_sb = xp.tile([P, KD, 512], DT16, tag="x", name="x0_sb")[:, :, :w0]
                if bulk_dma:
                    if lead_split:
                        # DMAs are consumed roughly in issue order: put the
                        # pieces the first matmuls need first (x k-piece 1,
                        # wg/wu first h-piece), then the rest.
                        ksz = KD // lead_split
                        bounds = [H * i // hsplit for i in range(hsplit + 1)]
                        bounds = [(b // P) * P for b in bounds]
                        bounds[-1] = H
                        bounds = [0, 2 * P] + [b for b in bounds[1:] if b > 2 * P]
                        nc.sync.dma_start(
                            x0_sb[:, 0:ksz, :], xt_r[:, 0:ksz, ds(start0, w0)]
                        )
                        nc.sync.dma_start(wg_sb[:, :, : bounds[1]], wg_rr[:, :, : bounds[1]])
                        nc.sync.dma_start(wu_sb[:, :, : bounds[1]], wu_rr[:, :, : bounds[1]])
                        for k0 in range(ksz, KD, ksz):
                            nc.sync.dma_start(
                                x0_sb[:, k0 : k0 + ksz, :],
                                xt_r[:, k0 : k0 + ksz, ds(start0, w0)],
                            )
                        for h0, h1 in zip(bounds[1:-1], bounds[2:]):
                            nc.sync.dma_start(wg_sb[:, :, h0:h1], wg_rr[:, :, h0:h1])
                            nc.sync.dma_start(wu_sb[:, :, h0:h1], wu_rr[:, :, h0:h1])
                    else:
                        nc.sync.dma_start(x0_sb[:], xt_r[:, :, ds(start0, w0)])
                        if hsplit > 1:
                            # split along H so early h-tiles' weights land first;
                            # pieces stay >=1.4MB for multi-queue DMA fanout
                            bounds = [H * i // hsplit for i in range(hsplit + 1)]
                            bounds = [(b // P) * P for b in bounds]
                            bounds[-1] = H
                            for h0, h1 in zip(bounds[:-1], bounds[1:]):
                                nc.sync.dma_start(wg_sb[:, :, h0:h1], wg_rr[:, :, h0:h1])
                                nc.sync.dma_start(wu_sb[:, :, h0:h1], wu_rr[:, :, h0:h1])
                        else:
                            nc.sync.dma_start(wg_sb[:], wg_rr)
                            nc.sync.dma_start(wu_sb[:], wu_rr)
                else:
                    for k in range(KD):
                        nc.sync.dma_start(x0_sb[:, k, :], xt_r[:, k, ds(start0, w0)])
                        nc.sync.dma_start(wg_sb[:, k, :], wg_rr[:, k, :])
                        nc.sync.dma_start(wu_sb[:, k, :], wu_rr[:, k, :])
                wd_sb = wdp.tile([P, KH, D], DT16, tag="wd")
                wd_rr = wd_d.rearrange("(ho p) d -> p ho d", p=P)
                if bulk_dma:
                    nc.sync.dma_start(wd_sb[:], wd_rr)
                elif not wd_late:
                    for h in range(KH):
                        nc.sync.dma_start(wd_sb[:, h, :], wd_rr[:, h, :])

                for ci, (start, w) in enumerate(chunk_list):
                    if wd_late and ci == 1:
                        for h in range(KH):
                            nc.sync.dma_start(wd_sb[:, h, :], wd_rr[:, h, :])
                    if ci == 0:
                        x_sb = x0_sb
                    else:
                        x_sb = xp.tile([P, KD, 512], DT16, tag="x", name="x_sb")[:, :, :w]
                        if bulk_dma:
                            nc.sync.dma_start(x_sb, xt_r[:, :, ds(start, w)])
                        else:
                            for k in range(KD):
                                nc.sync.dma_start(x_sb[:, k, :], xt_r[:, k, ds(start, w)])

                    aT = apool.tile([P, KH, 512], DT16, tag="a", name="aT")[:, :, :w]
                    for h in range(KH):
                        pg = psgu.tile([P, 512], F32, tag="psg", name="pg")[:, :w]
                        pu = psgu2.tile([P, 512], F32, tag="psu", name="pu")[:, :w]
                        if gu_interleave:
                            for k in range(KD):
                                nc.tensor.matmul(
                                    pg, wg_sb[:, k, h * P : (h + 1) * P], x_sb[:, k, :],
                                    start=(k == 0), stop=(k == KD - 1),
                                )
                                nc.tensor.matmul(
                                    pu, wu_sb[:, k, h * P : (h + 1) * P], x_sb[:, k, :],
                                    start=(k == 0), stop=(k == KD - 1),
                                )
                        else:
                            for k in range(KD):
                                nc.tensor.matmul(
                                    pg, wg_sb[:, k, h * P : (h + 1) * P], x_sb[:, k, :],
                                    start=(k == 0), stop=(k == KD - 1),
                                )
                            for k in range(KD):
                                nc.tensor.matmul(
                                    pu, wu_sb[:, k, h * P : (h + 1) * P], x_sb[:, k, :],
                                    start=(k == 0), stop=(k == KD - 1),
                                )
                        nc.scalar.activation(
                            aT[:, h, :], pg, mybir.ActivationFunctionType.Silu
                        )
                        nc.vector.tensor_tensor(
                            aT[:, h, :], aT[:, h, :], pu, mybir.AluOpType.mult
                        )

                    for g in range((w + P - 1) // P):
                        gw = min(P, w - g * P)  # final group may be 64 wide
                        gg = (start + g * P) // P
                        out_sb = opool.tile([P, 4, 512], DT16 if y16 else F32, tag="o", name="out_sb")
                        for d4 in range(4):
                            py = psy.tile([P, 512], F32, tag="psy", name="py")[:gw]
                            for h in range(KH):
                                nc.tensor.matmul(
                                    py,
                                    aT[:, h, g * P : g * P + gw],
                                    wd_sb[:, h, d4 * 512 : (d4 + 1) * 512],
                                    start=(h == 0),
                                    stop=(h == KH - 1),
                                )
                            nc.vector.tensor_scalar_mul(
                                out_sb[:gw, d4, :], py, cw_sb[:gw, gg : gg + 1]
                            )
                        se = nc.scalar if store_eng == "scalar" else nc.sync
                        if ysplit:
                            for d4 in range(4):
                                se.dma_start(
                                    y_r[:gw, gg, d4 * 512 : (d4 + 1) * 512],
                                    out_sb[:gw, d4, :],
                                )
                        else:
                            se.dma_start(y_r[:gw, gg, :], out_sb[:gw])

    nc.compile()
    return nc


def build_kernel_fp8(C, S, repeat=1, xb=2, ab=2, ob=2, pgu=3, pyb=2, y16=True,
                     hsplit=4, order="term"):
    """fp8 gate/up variant: x and Wg/Wu arrive as e4m3 (hi, lo) pairs; the
    gate/up matmuls run as 3-term DoubleRow fp8 (xh@Wh + xl@Wh + xh@Wl) at
    2x contraction per instruction. Scales: x*1, W*32 -> psum = 32*(x@W.T);
    silu applies scale=1/32; aT = silu(g) * (32*u) = 32*act; Wd is host-
    divided by 32 so the down matmul and cw path are unchanged."""
    TT = C + S
    assert C % P == 0 and S % P == 0
    KP = KD // 2  # 8 DoubleRow k-pairs

    nc = bacc.Bacc(
        "TRN2",
        target_bir_lowering=False,
        debug=False,
        enable_asserts=False,
        num_devices=8,
    )

    xth = nc.dram_tensor("xth", [D, TT], F8, kind="ExternalInput").ap()
    xtl = nc.dram_tensor("xtl", [D, TT], F8, kind="ExternalInput").ap()
    wts = {}
    for pref in ("e", "s"):
        wts[pref] = (
            nc.dram_tensor(f"wgh_{pref}", [D, H], F8, kind="ExternalInput").ap(),
            nc.dram_tensor(f"wgl_{pref}", [D, H], F8, kind="ExternalInput").ap(),
            nc.dram_tensor(f"wuh_{pref}", [D, H], F8, kind="ExternalInput").ap(),
            nc.dram_tensor(f"wul_{pref}", [D, H], F8, kind="ExternalInput").ap(),
            nc.dram_tensor(f"wd_{pref}", [H, D], DT16, kind="ExternalInput").ap(),
        )
    cw = nc.dram_tensor("cw", [P, TT // P], F32, kind="ExternalInput").ap()
    y = nc.dram_tensor("y", [TT, D], DT16 if y16 else F32, kind="ExternalOutput").ap()

    xth_r = xth.rearrange("(ko p) t -> p ko t", p=P)   # [128, 16, TT]
    xtl_r = xtl.rearrange("(ko p) t -> p ko t", p=P)
    y_r = y.rearrange("(g p) d -> p g d", p=P)         # [128, TT/128, 2048]

    phases = [("s", 0, S), ("e", S, C)]  # match _prepare's shared-first layout
    DR = mybir.MatmulPerfMode.DoubleRow

    with tile.TileContext(nc) as tc:
        with (
            tc.tile_pool(name="wghp", bufs=1) as wghp,
            tc.tile_pool(name="wglp", bufs=1) as wglp,
            tc.tile_pool(name="wuhp", bufs=1) as wuhp,
            tc.tile_pool(name="wulp", bufs=1) as wulp,
            tc.tile_pool(name="wdp", bufs=1) as wdp,
            tc.tile_pool(name="xhp", bufs=xb) as xhp,
            tc.tile_pool(name="xlp", bufs=xb) as xlp,
            tc.tile_pool(name="ap", bufs=ab) as apool,
            tc.tile_pool(name="op", bufs=ob) as opool,
            tc.tile_pool(name="cp", bufs=1) as cpool,
            tc.tile_pool(name="psgu", bufs=pgu, space="PSUM") as psgu,
            tc.tile_pool(name="psgu2", bufs=pgu, space="PSUM") as psgu2,
            tc.tile_pool(name="psy", bufs=pyb, space="PSUM") as psy,
        ):
            cw_sb = cpool.tile([P, TT // P], F32)
            nc.sync.dma_start(cw_sb[:], cw)

            for pref, base, count in phases * repeat:
                if count == 0:
                    continue
                wgh_d, wgl_d, wuh_d, wul_d, wd_d = wts[pref]
                wgh_sb = wghp.tile([P, KD, H], F8, tag="wgh")
                wgl_sb = wglp.tile([P, KD, H], F8, tag="wgl")
                wuh_sb = wuhp.tile([P, KD, H], F8, tag="wuh")
                wul_sb = wulp.tile([P, KD, H], F8, tag="wul")
                w_rr = [t.rearrange("(ko p) h -> p ko h", p=P)
                        for t in (wgh_d, wgl_d, wuh_d, wul_d)]
                w_sb = [wgh_sb, wgl_sb, wuh_sb, wul_sb]
                chunk_list = _chunks(count, base)
                start0, w0 = chunk_list[0]
                xh0_sb = xhp.tile([P, KD, 512], F8, tag="xh", name="xh0_sb")[:, :, :w0]
                xl0_sb = xlp.tile([P, KD, 512], F8, tag="xl", name="xl0_sb")[:, :, :w0]
                nc.sync.dma_start(xh0_sb[:], xth_r[:, :, ds(start0, w0)])
                nc.sync.dma_start(xl0_sb[:], xtl_r[:, :, ds(start0, w0)])
                bounds = [H * i // hsplit for i in range(hsplit + 1)]
                bounds = [(b // P) * P for b in bounds]
                bounds[-1] = H
                for h0, h1 in zip(bounds[:-1], bounds[1:]):
                    for sb, rr in zip(w_sb, w_rr):
                        nc.sync.dma_start(sb[:, :, h0:h1], rr[:, :, h0:h1])
                wd_sb = wdp.tile([P, KH, D], DT16, tag="wd")
                wd_rr = wd_d.rearrange("(ho p) d -> p ho d", p=P)
                nc.sync.dma_start(wd_sb[:], wd_rr)

                for ci, (start, w) in enumerate(chunk_list):
                    if ci == 0:
                        xh_sb, xl_sb = xh0_sb, xl0_sb
                    else:
                        xh_sb = xhp.tile([P, KD, 512], F8, tag="xh", name="xh_sb")[:, :, :w]
                        xl_sb = xlp.tile([P, KD, 512], F8, tag="xl", name="xl_sb")[:, :, :w]
                        nc.sync.dma_start(xh_sb, xth_r[:, :, ds(start, w)])
                        nc.sync.dma_start(xl_sb, xtl_r[:, :, ds(start, w)])

                    aT = apool.tile([P, KH, 512], DT16, tag="a", name="aT")[:, :, :w]
                    for h in range(KH):
                        pg = psgu.tile([P, 512], F32, tag="psg", name="pg")[:, :w]
                        pu = psgu2.tile([P, 512], F32, tag="psu", name="pu")[:, :w]
                        hs = slice(h * P, (h + 1) * P)
                        if order == "term":
                            # 3 DoubleRow sweeps per matrix, g/u interleaved
                            terms_g = [(xh_sb, wgh_sb), (xl_sb, wgh_sb), (xh_sb, wgl_sb)]
                            terms_u = [(xh_sb, wuh_sb), (xl_sb, wuh_sb), (xh_sb, wul_sb)]
                            for t in range(3):
                                xg, wg_ = terms_g[t]
                                xu, wu_ = terms_u[t]
                                for kp in range(KP):
                                    ks = slice(2 * kp, 2 * kp + 2)
                                    nc.tensor.matmul(
                                        pg, wg_[:, ks, hs], xg[:, ks, :],
                                        start=(t == 0 and kp == 0),
                                        stop=(t == 2 and kp == KP - 1),
                                        perf_mode=DR,
                                    )
                                    nc.tensor.matmul(
                                        pu, wu_[:, ks, hs], xu[:, ks, :],
                                        start=(t == 0 and kp == 0),
                                        stop=(t == 2 and kp == KP - 1),
                                        perf_mode=DR,
                                    )
                        else:  # "kp": explicit ldweights, stationary reused
                            for kp in range(KP):
                                ks = slice(2 * kp, 2 * kp + 2)
                                for ps, whi, wlo in (
                                    (pg, wgh_sb, wgl_sb),
                                    (pu, wuh_sb, wul_sb),
                                ):
                                    nc.tensor.ldweights(whi[:, ks, hs], perf_mode=DR)
                                    for i, xs in enumerate((xh_sb, xl_sb)):
                                        mm = nc.tensor.matmul(
                                            ps, whi[:, ks, hs], xs[:, ks, :],
                                            start=(kp == 0 and i == 0),
                                            stop=False,
                                            perf_mode=DR,
                                        )
                                        mm.ldweights = False
                                    nc.tensor.ldweights(wlo[:, ks, hs], perf_mode=DR)
                                    mm = nc.tensor.matmul(
                                        ps, wlo[:, ks, hs], xh_sb[:, ks, :],
                                        start=False,
                                        stop=(kp == KP - 1),
                                        perf_mode=DR,
                                    )
                                    mm.ldweights = False
                        nc.scalar.activation(
                            aT[:, h, :], pg, mybir.ActivationFunctionType.Silu,
                            scale=1.0 / SW,
                        )
                        nc.vector.tensor_tensor(
                            aT[:, h, :], aT[:, h, :], pu, mybir.AluOpType.mult
                        )

                    for g in range(w // P):
                        gg = (start + g * P) // P
                        out_sb = opool.tile([P, 4, 512], DT16 if y16 else F32, tag="o", name="out_sb")
                        for d4 in range(4):
                            py = psy.tile([P, 512], F32, tag="psy", name="py")
                            for h in range(KH):
                                nc.tensor.matmul(
                                    py,
                                    aT[:, h, g * P : (g + 1) * P],
                                    wd_sb[:, h, d4 * 512 : (d4 + 1) * 512],
                                    start=(h == 0),
                                    stop=(h == KH - 1),
                                )
                            nc.vector.tensor_scalar_mul(
                                out_sb[:, d4, :], py, cw_sb[:, gg : gg + 1]
                            )
                        nc.sync.dma_start(y_r[:, gg, :], out_sb[:])

    nc.compile()
    return nc


def _route(x_flat, gate_w, expert_bias):
    """Replicate the reference router in numpy (fp32)."""
    N = x_flat.shape[0]
    logits = x_flat @ gate_w.T                       # [N, E]
    m = logits.max(-1, keepdims=True)
    p = np.exp(logits - m)
    p /= p.sum(-1, keepdims=True)
    biased = logits + expert_bias
    rows = np.arange(N)
    i1 = biased.argmax(-1)
    b2 = biased.copy()
    b2[rows, i1] = -np.inf
    i2 = b2.argmax(-1)
    w1 = p[rows, i1]
    w2 = p[rows, i2]
    s = w1 + w2
    return i1, i2, w1 / s, w2 / s


def _prepare(inputs):
    x = np.asarray(inputs["x"], dtype=np.float32)
    B, S_, D_ = x.shape
    assert D_ == D
    x_flat = x.reshape(-1, D)
    N = x_flat.shape[0]
    S0 = N // 8

    i1, i2, w1, w2 = _route(
        x_flat,
        np.asarray(inputs["gate_w"], dtype=np.float32),
        np.asarray(inputs["expert_bias"], dtype=np.float32),
    )

    idx_lists = []
    w_lists = []
    for e in range(E):
        m1 = i1 == e
        m2 = i2 == e
        idx = np.nonzero(m1 | m2)[0]
        w = np.where(m1[idx], w1[idx], w2[idx]).astype(np.float32)
        idx_lists.append(idx)
        w_lists.append(w)

    maxc = max(len(ix) for ix in idx_lists)
    C = _ceil(maxc, 64)  # PE supports 64-wide output tiles; saves pad work
    TT = C + S0
    TT_pad = _ceil(TT, P)

    bf = np.float16
    Wg = np.asarray(inputs["Wg"], dtype=np.float32)
    Wu = np.asarray(inputs["Wu"], dtype=np.float32)
    Wd = np.asarray(inputs["Wd"], dtype=np.float32)
    fp8 = FP8_GU

    def split8(a):
        """fp32 array -> (hi, lo) e4m3 pair with hi + lo ~= a."""
        hi = a.astype(E4M3)
        lo = (a - hi.astype(np.float32)).astype(E4M3)
        return hi, lo

    if fp8:
        xfh, xfl = split8(x_flat)

    def gu_weights(wgT, wuT, wdT, pref):
        """Per-phase weight dict from [D,H] gate/up and [H,D] down (fp32)."""
        if not fp8:
            return {
                f"wg_{pref}": np.ascontiguousarray(wgT.T).astype(bf),
                f"wu_{pref}": np.ascontiguousarray(wuT.T).astype(bf),
                f"wd_{pref}": np.ascontiguousarray(wdT.T).astype(bf),
            }
        wgh, wgl = split8(np.ascontiguousarray(wgT.T) * SW)
        wuh, wul = split8(np.ascontiguousarray(wuT.T) * SW)
        return {
            f"wgh_{pref}": wgh, f"wgl_{pref}": wgl,
            f"wuh_{pref}": wuh, f"wul_{pref}": wul,
            f"wd_{pref}": (np.ascontiguousarray(wdT.T) / SW).astype(bf),
        }

    sw = gu_weights(
        np.asarray(inputs["Ws_g"], np.float32),
        np.asarray(inputs["Ws_u"], np.float32),
        np.asarray(inputs["Ws_d"], np.float32),
        "s",
    )

    in_maps = []
    idx_pad = np.empty((E, C), dtype=np.int64)
    for e in range(E):
        idx = idx_lists[e]
        pad = np.full(C - len(idx), N, dtype=np.int64)  # N -> dummy row
        idx_pad[e] = np.concatenate([idx, pad])
        gather_idx = np.concatenate([idx, np.zeros(C - len(idx), np.int64)])

        # layout: shared tokens first [0:S0), expert tokens [S0:S0+C),
        # dram padding [TT:TT_pad) (never read/written by the device)
        cwv = np.zeros(TT_pad, dtype=np.float32)
        cwv[:S0] = 1.0
        cwv[S0 : S0 + len(idx)] = w_lists[e]
        cwv = np.ascontiguousarray(cwv.reshape(TT_pad // P, P).T)

        im = {"cw": cwv}
        if fp8:
            assert TT == TT_pad, "fp8 path requires 128-aligned TT"
            for nm, src in (("xth", xfh), ("xtl", xfl)):
                xt = np.zeros((D, TT_pad), dtype=E4M3)
                xt[:, :S0] = src[e * S0 : (e + 1) * S0].T
                xt[:, S0 : S0 + C] = src[gather_idx].T
                im[nm] = xt
        else:
            xt = np.zeros((D, TT_pad), dtype=bf)
            xt[:, :S0] = x_flat[e * S0 : (e + 1) * S0].T
            xt[:, S0 : S0 + C] = x_flat[gather_idx].T
            im["xt"] = xt
        im.update(gu_weights(Wg[e], Wu[e], Wd[e], "e"))
        im.update(sw)
        in_maps.append(im)
    return x, in_maps, idx_pad, C, S0, N


def _combine(x_shape, results, idx_pad, C, S0, N):
    acc = np.zeros((N + 1, D), dtype=np.float32)
    for e in range(E):
        ye = results[e]["y"]
        acc[e * S0 : (e + 1) * S0] += ye[:S0]
        acc[idx_pad[e]] += ye[S0 : S0 + C]
    return acc[:N].reshape(x_shape)


def kernel(**inputs) -> np.ndarray:
    x, in_maps, idx_pad, C, S0, N = _prepare(inputs)
    nc = build_kernel(C, S0)
    res = run_bass_kernel_spmd(nc, in_maps, core_ids=list(range(8)))
    return _combine(x.shape, [res.results[e] for e in range(E)], idx_pad, C, S0, N)

